# revision 11
# baseline (speedup 1.0000x reference)
"""Modulated 3x3 conv (StyleGAN2-style, groups=B) on 8 trn2 NeuronCores.

Sharding: data-parallel over (batch B=4) x (image half H/2), 8 shards.
Each core computes a full 64->64 channel 3x3 conv over a 256-row half of
one sample's 512x512 image.

All style/demodulation math is folded on the host: the shipped conv
weights are already modulated (s = affine(w)+1), demodulated
(d = rsqrt(sum (s*W)^2 + eps)) and transposed into the stationary-tile
layout the tensor engine consumes. Host also pre-pads and relayouts x
into a "slot" layout and quantizes it, so device DMAs move at most half
the fp32 byte count, and the output returns as bf16.

Slot layout: partition half h in {0,1} holds rows of parity (h==1 ?
even : odd); local input row R (in -1..256, incl. 1-row halo each side)
lives at free slot ceil(R/2), i.e. slot m holds rows (2m-1, 2m). An
output row-pair q (out rows 2q, 2q+1) needs exactly slots (q, q+1) =
input rows 2q-1..2q+2. Each row has a zero pad column on both sides
(514 cols/slot) so the 3 kw taps are column shifts.

Matmul schemes into one PSUM bank per pair ([128, 512] = 2 out rows x
64 cout x 512 cols; out partition = tau*64+co -> out row 2q+tau), with
lhsT quadrant (h, tau) holding W[kh = h + 2i - tau] (i = A/B slot index;
invalid kh -> zero block):
  tf32c: 6 fp32r matmuls (A/B slot x 3 kw), x shipped bf16 and cast to
         fp32r by the DMA load (halves HBM read bytes; PE streams TF32
         at 1 col/cycle). x streamed through a chunked tile pool.
  bf16 : same 6 matmuls on bf16 data, x resident in SBUF.
  fp8dr: 9 DoubleRow matmuls (3 terms x 3 kw): Wh*xh + Wh*xl + Wl*xh,
         hi/lo fp8 split of x and the (scaled by S) weights; PSUM
         evicted with a 1/S constant scale.
"""

import numpy as np
import ml_dtypes

import concourse.bacc as bacc
import concourse.mybir as mybir
import concourse.tile as tile
from concourse.bass_utils import run_bass_kernel_spmd

B, CIN, COUT, L, H, W = 4, 64, 64, 512, 512, 512
KH = KW = 3
N_CORES = 8
HALF = H // 2  # 256 output rows per core
NPAIR = HALF // 2  # 128 output row pairs per core
NSLOT = NPAIR + 1  # 129 input slots per partition half
ROWB = W + 2  # per-row SBUF columns (1 zero pad each side)
EPS = 1e-8
F32 = mybir.dt.float32
F32R = mybir.dt.float32r
BF16 = mybir.dt.bfloat16
FP8 = mybir.dt.float8e4
NPF8 = ml_dtypes.float8_e4m3

MODE = "tf32f"  # "tf32f" | "tf32c" | "bf16" | "fp16" | "fp8dr"
ORDER = "wo"  # "wo": weights-outer (reused across a group) | "wi": per-pair
CAST_ENGINE = "gpsimd"  # bf16->f32r cast loads require SWDGE (gpsimd)
WSCALE = 32.0  # fp8 weight scale (1/WSCALE folded into eviction)
OG = 8  # pairs per output group
NOG = NPAIR // OG
# x load chunks in slots: fine-grained at the start so compute ramps early
CHUNKS = [(0, 2), (2, 3), (5, 4), (9, 8), (17, 8), (25, 8)]
_c = 33
while _c < NSLOT:
    n = min(8, NSLOT - _c)
    CHUNKS.append((_c, n))
    _c += n
XCHUNK = 8  # physical slots per x-pool tile (all chunks share one size)
XBUFS = 8
PREFETCH = 10  # slots of lookahead beyond current group's needs

_CACHE = {}


def _mode_cfg():
    if MODE == "tf32c":
        return dict(nmm=6, planes=1, xdt=BF16, sdt=F32R, resident=False)
    if MODE == "tf32f":
        return dict(nmm=6, planes=1, xdt=F32R, sdt=F32R, resident=False)
    if MODE == "bf16":
        return dict(nmm=6, planes=1, xdt=BF16, sdt=BF16, resident=True)
    if MODE == "fp16":
        F16 = mybir.dt.float16
        return dict(nmm=6, planes=1, xdt=F16, sdt=F16, resident=True)
    return dict(nmm=9, planes=2, xdt=FP8, sdt=FP8, resident=True)


def _build_nc(reps=1):
    cfg = _mode_cfg()
    nc = bacc.Bacc("TRN2", target_bir_lowering=False, debug=False)
    x1 = nc.dram_tensor("x1", [128, NSLOT * ROWB], cfg["xdt"], kind="ExternalInput")
    x2 = (
        nc.dram_tensor("x2", [128, NSLOT * ROWB], cfg["xdt"], kind="ExternalInput")
        if MODE == "fp8dr"
        else None
    )
    wtl = nc.dram_tensor(
        "wtl", [cfg["nmm"], 128, cfg["planes"] * 128], cfg["sdt"],
        kind="ExternalInput",
    )
    # out layout: [parity, co, row-pair, w]; out[t, co, q, :] = out row 2q+t
    out = nc.dram_tensor("out", [2, COUT, NPAIR, W], BF16, kind="ExternalOutput")

    with tile.TileContext(nc) as tc:
        for _ in range(reps):
            _emit(tc, cfg, x1, x2, wtl, out)
    nc.compile()
    return nc


def _emit(tc, cfg, x1, x2, wtl, out):
    nc = tc.nc
    nmm, planes, sdt = cfg["nmm"], cfg["planes"], cfg["sdt"]
    const = tc.alloc_tile_pool(name="const", bufs=1)
    opool = tc.alloc_tile_pool(name="og", bufs=3)
    pmain = tc.alloc_tile_pool(name="pmain", bufs=8, space="PSUM")

    WTILES = []
    for k in range(nmm):
        t = const.tile([128, planes * 128], sdt, tag=f"WT{k}")
        nc.sync.dma_start(t[:], wtl[k, :, :])
        WTILES.append(t)

    x1v_d = x1.rearrange("p (s c) -> p s c", c=ROWB)
    x2v_d = x2.rearrange("p (s c) -> p s c", c=ROWB) if x2 is not None else None
    li = [0]  # next chunk index
    loaded = [0]  # slots fully loaded

    if cfg["resident"]:
        xt1 = const.tile([128, NSLOT * ROWB], sdt, tag="x1")
        xv1 = xt1[:].rearrange("p (s c) -> p s c", c=ROWB)
        xv2 = None
        if MODE == "fp8dr":
            xt2 = const.tile([128, NSLOT * ROWB], sdt, tag="x2")
            xv2 = xt2[:].rearrange("p (s c) -> p s c", c=ROWB)

        def ensure_loaded(slot_needed):
            while li[0] < len(CHUNKS) and loaded[0] < min(slot_needed, NSLOT):
                s0, n = CHUNKS[li[0]]
                nc.sync.dma_start(xv1[:, s0 : s0 + n, :], x1v_d[:, s0 : s0 + n, :])
                if MODE == "fp8dr":
                    nc.sync.dma_start(
                        xv2[:, s0 : s0 + n, :], x2v_d[:, s0 : s0 + n, :]
                    )
                li[0] += 1
                loaded[0] = s0 + n

        def slot_rhs(q, kw):
            return xv1[:, q, kw : kw + 512]

        def pair_rhs(q, kw, term):
            return (xv1 if term != 1 else xv2)[:, q : q + 2, kw : kw + 512]

    else:
        xpool = tc.alloc_tile_pool(name="xg", bufs=XBUFS)
        xslot = {}  # slot -> (tile-view, idx)
        eng = nc.gpsimd if CAST_ENGINE == "gpsimd" else nc.sync

        def ensure_loaded(slot_needed):
            while li[0] < len(CHUNKS) and loaded[0] < min(slot_needed, NSLOT):
                s0, n = CHUNKS[li[0]]
                t = xpool.tile([128, XCHUNK * ROWB], sdt, tag="xg")
                v = t[:].rearrange("p (s c) -> p s c", c=ROWB)
                eng.dma_start(v[:, 0:n, :], x1v_d[:, s0 : s0 + n, :])
                for i in range(n):
                    xslot[s0 + i] = (v, i)
                li[0] += 1
                loaded[0] = s0 + n

        def slot_rhs(q, kw):
            v, i = xslot[q]
            return v[:, i, kw : kw + 512]

        def pair_rhs(q, kw, term):
            raise NotImplementedError

    outv = out.rearrange("t co q w -> (t co) q w")  # [128, NPAIR, W]
    scale = 1.0 / WSCALE if MODE == "fp8dr" else 1.0

    def evict(ogv, j, ps):
        # alternate DVE / ACT so neither engine is critical
        if j % 2 == 0:
            nc.vector.tensor_scalar_mul(ogv[:, j, :], ps[:], scale)
        else:
            nc.scalar.activation(
                ogv[:, j, :],
                ps[:],
                mybir.ActivationFunctionType.Copy,
                scale=scale,
            )

    def compute_group(g):
        og = opool.tile([128, OG * 512], BF16, tag="og")
        ogv = og[:].rearrange("p (j w) -> p j w", w=512)
        if MODE != "fp8dr" and ORDER == "wo":
            ps = [
                pmain.tile([128, 512], F32, tag="ps", name=f"ps_{g}_{j}")
                for j in range(OG)
            ]
            for k in range(nmm):
                i, kw = divmod(k, KW)
                for j in range(OG):
                    nc.tensor.matmul(
                        ps[j][:],
                        WTILES[k][:],
                        slot_rhs(g * OG + j + i, kw),
                        start=(k == 0),
                        stop=(k == nmm - 1),
                    )
            for j in range(OG):
                evict(ogv, j, ps[j])
        else:
            for j in range(OG):
                q = g * OG + j
                ps = pmain.tile([128, 512], F32, tag="ps")
                if MODE == "fp8dr":
                    k = 0
                    for t in range(3):  # Wh*xh, Wh*xl, Wl*xh
                        for kw in range(KW):
                            nc.tensor.matmul(
                                ps[:],
                                WTILES[t * 3 + kw][:].rearrange(
                                    "p (i m) -> p i m", i=2
                                ),
                                pair_rhs(q, kw, t),
                                start=(k == 0),
                                stop=(k == nmm - 1),
                                perf_mode=mybir.MatmulPerfMode.DoubleRow,
                            )
                            k += 1
                else:
                    k = 0
                    for i in range(2):  # A (slot q), B (slot q+1)
                        for kw in range(KW):
                            nc.tensor.matmul(
                                ps[:],
                                WTILES[i * 3 + kw][:],
                                slot_rhs(q + i, kw),
                                start=(k == 0),
                                stop=(k == nmm - 1),
                            )
                            k += 1
                evict(ogv, j, ps)
        half_g = OG // 2
        nc.sync.dma_start(outv[:, OG * g : OG * g + half_g, :], ogv[:, 0:half_g, :])
        nc.sync.dma_start(
            outv[:, OG * g + half_g : OG * (g + 1), :], ogv[:, half_g:OG, :]
        )

    for g in range(NOG):
        ensure_loaded(OG * (g + 1) + 1 + PREFETCH)
        compute_group(g)

    pools = [pmain, opool, const]
    if not cfg["resident"]:
        pools.insert(0, xpool)
    for p in pools:
        p.release()


def _get_nc(reps=1):
    if reps not in _CACHE:
        _CACHE[reps] = _build_nc(reps)
    return _CACHE[reps]


def _rn8(a):
    return a.astype(NPF8)


def _host_weights(weight, w, affine_w, affine_b):
    """Per-sample modulated+demodulated stationary tiles, quantized."""
    weight = np.asarray(weight, dtype=np.float64)
    w = np.asarray(w, dtype=np.float64)
    affine_w = np.asarray(affine_w, dtype=np.float64)
    affine_b = np.asarray(affine_b, dtype=np.float64)
    cfg = _mode_cfg()
    nmm, planes = cfg["nmm"], cfg["planes"]
    npdt = {
        F32R: np.float32,
        BF16: ml_dtypes.bfloat16,
        mybir.dt.float16: np.float16,
        FP8: NPF8,
    }[cfg["sdt"]]
    per_sample = []
    for b in range(B):
        s = w[b] @ affine_w.T + affine_b + 1.0  # [CIN]
        wm = s[None, :, None, None] * weight  # [co, ci, kh, kw]
        d = 1.0 / np.sqrt((wm * wm).sum(axis=(1, 2, 3)) + EPS)  # [co]
        wd = (d[:, None, None, None] * wm).transpose(1, 0, 2, 3)  # [ci, co, kh, kw]
        tiles = np.zeros((nmm, 128, planes * 128), dtype=npdt)
        if MODE != "fp8dr":
            wq = wd.astype(np.float32)
            for i in range(2):
                for kw in range(KW):
                    tl = tiles[i * 3 + kw]
                    for h in range(2):
                        for tau in range(2):
                            kh = h + 2 * i - tau
                            if 0 <= kh < KH:
                                tl[
                                    h * 64 : h * 64 + 64, tau * 64 : tau * 64 + 64
                                ] = wq[:, :, kh, kw]
        else:
            wq = (wd * WSCALE).astype(np.float32)
            wh = _rn8(wq)
            wl = _rn8(wq - wh.astype(np.float32))
            tv = tiles.reshape(nmm, 128, 2, 128)
            for t in range(3):
                w8 = wh if t < 2 else wl
                for kw in range(KW):
                    tl = tv[t * 3 + kw]
                    for h in range(2):
                        for i in range(2):
                            for tau in range(2):
                                kh = h + 2 * i - tau
                                if 0 <= kh < KH:
                                    tl[
                                        h * 64 : h * 64 + 64,
                                        i,
                                        tau * 64 : tau * 64 + 64,
                                    ] = w8[:, :, kh, kw]
        per_sample.append(tiles)
    return per_sample


def _host_x(x):
    """Slot-layout, padded, quantized per-core x tensors."""
    x = np.asarray(x, dtype=np.float32)
    cfg = _mode_cfg()
    npdt = {
        F32R: np.float32,
        BF16: ml_dtypes.bfloat16,
        mybir.dt.float16: np.float16,
        FP8: NPF8,
    }[cfg["xdt"]]
    shards = []
    for core in range(N_CORES):
        b, half = divmod(core, 2)
        h0 = half * HALF
        xsh = np.zeros((CIN, 2 * NSLOT, W), dtype=np.float32)
        lo, hi = h0 - 1, h0 + HALF + 1  # global rows [lo, hi)
        clo, chi = max(lo, 0), min(hi, H)
        xsh[:, clo - lo : chi - lo, :] = x[b, :, clo:chi, :]
        # xsh index idx = local row R + 1, R in -1..256
        # slot m: h=0 -> R=2m-1 (idx 2m), h=1 -> R=2m (idx 2m+1)
        xs = np.zeros((2, CIN, NSLOT, ROWB), dtype=np.float32)
        xs[0, :, :, 1:513] = xsh[:, 0 : 2 * NSLOT : 2, :]
        xs[1, :, :, 1:513] = xsh[:, 1 : 2 * NSLOT : 2, :]
        xs = xs.reshape(128, NSLOT * ROWB)
        if MODE == "fp8dr":
            xh = _rn8(xs)
            xl = _rn8(xs - xh.astype(np.float32))
            shards.append((xh, xl))
        else:
            shards.append((xs.astype(npdt), None))
    return shards


def _shard_inputs(x, w, weight, affine_w, affine_b):
    wts = _host_weights(weight, w, affine_w, affine_b)
    xsh = _host_x(x)
    in_maps = []
    for core in range(N_CORES):
        b = core // 2
        m = {"x1": xsh[core][0], "wtl": wts[b]}
        if MODE == "fp8dr":
            m["x2"] = xsh[core][1]
        in_maps.append(m)
    return in_maps


def kernel(x, w, weight, affine_w, affine_b):
    nc = _get_nc()
    in_maps = _shard_inputs(x, w, weight, affine_w, affine_b)
    res = run_bass_kernel_spmd(nc, in_maps, list(range(N_CORES)))
    full = np.empty((B, COUT, H, W), dtype=np.float32)
    for core in range(N_CORES):
        b, half = divmod(core, 2)
        o2 = res.results[core]["out"].astype(np.float32)  # [2, COUT, NPAIR, W]
        full[b, :, half * HALF : (half + 1) * HALF, :] = (
            o2.transpose(1, 2, 0, 3).reshape(COUT, HALF, W)
        )
    return full


# revision 16
# speedup vs baseline: 1.1274x; 1.1274x over previous
"""Modulated 3x3 conv (StyleGAN2-style, groups=B) on 8 trn2 NeuronCores.

Sharding: data-parallel over (batch B=4) x (image half H/2), 8 shards.
Each core computes a full 64->64 channel 3x3 conv over a 256-row half of
one sample's 512x512 image.

All style/demodulation math is folded on the host: the shipped conv
weights are already modulated (s = affine(w)+1), demodulated
(d = rsqrt(sum (s*W)^2 + eps)) and transposed into the stationary-tile
layout the tensor engine consumes. Host also pre-pads and relayouts x
into a "slot" layout and quantizes it, so device DMAs move at most half
the fp32 byte count, and the output returns as bf16.

Slot layout: partition half h in {0,1} holds rows of parity (h==1 ?
even : odd); local input row R (in -1..256, incl. 1-row halo each side)
lives at free slot ceil(R/2), i.e. slot m holds rows (2m-1, 2m). An
output row-pair q (out rows 2q, 2q+1) needs exactly slots (q, q+1) =
input rows 2q-1..2q+2. Each row has a zero pad column on both sides
(514 cols/slot) so the 3 kw taps are column shifts.

Matmul schemes into one PSUM bank per pair ([128, 512] = 2 out rows x
64 cout x 512 cols; out partition = tau*64+co -> out row 2q+tau), with
lhsT quadrant (h, tau) holding W[kh = h + 2i - tau] (i = A/B slot index;
invalid kh -> zero block):
  tf32c: 6 fp32r matmuls (A/B slot x 3 kw), x shipped bf16 and cast to
         fp32r by the DMA load (halves HBM read bytes; PE streams TF32
         at 1 col/cycle). x streamed through a chunked tile pool.
  bf16 : same 6 matmuls on bf16 data, x resident in SBUF.
  fp8dr: 9 DoubleRow matmuls (3 terms x 3 kw): Wh*xh + Wh*xl + Wl*xh,
         hi/lo fp8 split of x and the (scaled by S) weights; PSUM
         evicted with a 1/S constant scale.
"""

import numpy as np
import ml_dtypes

import concourse.bacc as bacc
import concourse.mybir as mybir
import concourse.tile as tile
from concourse.bass_utils import run_bass_kernel_spmd

B, CIN, COUT, L, H, W = 4, 64, 64, 512, 512, 512
KH = KW = 3
N_CORES = 8
HALF = H // 2  # 256 output rows per core
NPAIR = HALF // 2  # 128 output row pairs per core
NSLOT = NPAIR + 1  # 129 input slots per partition half
ROWB = W + 2  # per-row SBUF columns (1 zero pad each side)
EPS = 1e-8
F32 = mybir.dt.float32
F32R = mybir.dt.float32r
BF16 = mybir.dt.bfloat16
FP8 = mybir.dt.float8e4
NPF8 = ml_dtypes.float8_e4m3

MODE = "tf32f"  # "tf32f" | "tf32c" | "bf16" | "fp16" | "fp8dr"
ORDER = "wo"  # "wo": weights-outer (reused across a group) | "wi": per-pair
CAST_ENGINE = "gpsimd"  # bf16->f32r cast loads require SWDGE (gpsimd)
LOAD_ENGINE = "gpsimd"  # x-load queue for non-cast modes: "sync" | "gpsimd"
EVICT = "dve"  # "dve": DVE-only | "alt": alternate DVE/ACT
STORES = True  # False: skip output DMAs (timing diagnostics only)
WSCALE = 32.0  # fp8 weight scale (1/WSCALE folded into eviction)
OG = 8  # pairs per output group
NOG = NPAIR // OG
# x load chunks in slots: fine-grained at the start so compute ramps early
CHUNKS = [(0, 2), (2, 3), (5, 4), (9, 8), (17, 8), (25, 8)]
_c = 33
while _c < NSLOT:
    n = min(8, NSLOT - _c)
    CHUNKS.append((_c, n))
    _c += n
XCHUNK = 8  # physical slots per x-pool tile (all chunks share one size)
XBUFS = 8
PREFETCH = 10  # slots of lookahead beyond current group's needs

_CACHE = {}


def _mode_cfg():
    if MODE == "tf32c":
        return dict(nmm=6, planes=1, xdt=BF16, sdt=F32R, resident=False)
    if MODE == "tf32f":
        return dict(nmm=6, planes=1, xdt=F32R, sdt=F32R, resident=False)
    if MODE == "bf16":
        return dict(nmm=6, planes=1, xdt=BF16, sdt=BF16, resident=True)
    if MODE == "fp16":
        F16 = mybir.dt.float16
        return dict(nmm=6, planes=1, xdt=F16, sdt=F16, resident=True)
    return dict(nmm=9, planes=2, xdt=FP8, sdt=FP8, resident=True)


def _build_nc(reps=1):
    cfg = _mode_cfg()
    nc = bacc.Bacc("TRN2", target_bir_lowering=False, debug=False)
    x1 = nc.dram_tensor("x1", [128, NSLOT * ROWB], cfg["xdt"], kind="ExternalInput")
    x2 = (
        nc.dram_tensor("x2", [128, NSLOT * ROWB], cfg["xdt"], kind="ExternalInput")
        if MODE == "fp8dr"
        else None
    )
    wtl = nc.dram_tensor(
        "wtl", [cfg["nmm"], 128, cfg["planes"] * 128], cfg["sdt"],
        kind="ExternalInput",
    )
    # out layout: [parity, co, row-pair, w]; out[t, co, q, :] = out row 2q+t
    out = nc.dram_tensor("out", [2, COUT, NPAIR, W], BF16, kind="ExternalOutput")

    with tile.TileContext(nc) as tc:
        for _ in range(reps):
            _emit(tc, cfg, x1, x2, wtl, out)
    nc.compile()
    return nc


def _emit(tc, cfg, x1, x2, wtl, out):
    nc = tc.nc
    nmm, planes, sdt = cfg["nmm"], cfg["planes"], cfg["sdt"]
    const = tc.alloc_tile_pool(name="const", bufs=1)
    opool = tc.alloc_tile_pool(name="og", bufs=3)
    pmain = tc.alloc_tile_pool(name="pmain", bufs=8, space="PSUM")

    WTILES = []
    for k in range(nmm):
        t = const.tile([128, planes * 128], sdt, tag=f"WT{k}")
        nc.sync.dma_start(t[:], wtl[k, :, :])
        WTILES.append(t)

    x1v_d = x1.rearrange("p (s c) -> p s c", c=ROWB)
    x2v_d = x2.rearrange("p (s c) -> p s c", c=ROWB) if x2 is not None else None
    li = [0]  # next chunk index
    loaded = [0]  # slots fully loaded

    if cfg["resident"]:
        xt1 = const.tile([128, NSLOT * ROWB], sdt, tag="x1")
        xv1 = xt1[:].rearrange("p (s c) -> p s c", c=ROWB)
        xv2 = None
        if MODE == "fp8dr":
            xt2 = const.tile([128, NSLOT * ROWB], sdt, tag="x2")
            xv2 = xt2[:].rearrange("p (s c) -> p s c", c=ROWB)

        def ensure_loaded(slot_needed):
            while li[0] < len(CHUNKS) and loaded[0] < min(slot_needed, NSLOT):
                s0, n = CHUNKS[li[0]]
                nc.sync.dma_start(xv1[:, s0 : s0 + n, :], x1v_d[:, s0 : s0 + n, :])
                if MODE == "fp8dr":
                    nc.sync.dma_start(
                        xv2[:, s0 : s0 + n, :], x2v_d[:, s0 : s0 + n, :]
                    )
                li[0] += 1
                loaded[0] = s0 + n

        def slot_rhs(q, kw):
            return xv1[:, q, kw : kw + 512]

        def pair_rhs(q, kw, term):
            return (xv1 if term != 1 else xv2)[:, q : q + 2, kw : kw + 512]

    else:
        xpool = tc.alloc_tile_pool(name="xg", bufs=XBUFS)
        xslot = {}  # slot -> (tile-view, idx)
        if MODE == "tf32c":
            eng = nc.gpsimd if CAST_ENGINE == "gpsimd" else nc.sync
        else:
            eng = nc.gpsimd if LOAD_ENGINE == "gpsimd" else nc.sync

        def ensure_loaded(slot_needed):
            while li[0] < len(CHUNKS) and loaded[0] < min(slot_needed, NSLOT):
                s0, n = CHUNKS[li[0]]
                t = xpool.tile([128, XCHUNK * ROWB], sdt, tag="xg")
                v = t[:].rearrange("p (s c) -> p s c", c=ROWB)
                eng.dma_start(v[:, 0:n, :], x1v_d[:, s0 : s0 + n, :])
                for i in range(n):
                    xslot[s0 + i] = (v, i)
                li[0] += 1
                loaded[0] = s0 + n

        def slot_rhs(q, kw):
            v, i = xslot[q]
            return v[:, i, kw : kw + 512]

        def pair_rhs(q, kw, term):
            raise NotImplementedError

    outv = out.rearrange("t co q w -> (t co) q w")  # [128, NPAIR, W]
    scale = 1.0 / WSCALE if MODE == "fp8dr" else 1.0

    def evict(ogv, j, ps):
        if EVICT == "alt" and j % 2 == 1:
            nc.scalar.activation(
                ogv[:, j, :],
                ps[:],
                mybir.ActivationFunctionType.Copy,
                scale=scale,
            )
        else:
            nc.vector.tensor_scalar_mul(ogv[:, j, :], ps[:], scale)

    def compute_group(g):
        og = opool.tile([128, OG * 512], BF16, tag="og")
        ogv = og[:].rearrange("p (j w) -> p j w", w=512)
        if MODE != "fp8dr" and ORDER == "wo":
            ps = [
                pmain.tile([128, 512], F32, tag="ps", name=f"ps_{g}_{j}")
                for j in range(OG)
            ]
            for k in range(nmm):
                i, kw = divmod(k, KW)
                for j in range(OG):
                    nc.tensor.matmul(
                        ps[j][:],
                        WTILES[k][:],
                        slot_rhs(g * OG + j + i, kw),
                        start=(k == 0),
                        stop=(k == nmm - 1),
                    )
            for j in range(OG):
                evict(ogv, j, ps[j])
        else:
            for j in range(OG):
                q = g * OG + j
                ps = pmain.tile([128, 512], F32, tag="ps")
                if MODE == "fp8dr":
                    k = 0
                    for t in range(3):  # Wh*xh, Wh*xl, Wl*xh
                        for kw in range(KW):
                            nc.tensor.matmul(
                                ps[:],
                                WTILES[t * 3 + kw][:].rearrange(
                                    "p (i m) -> p i m", i=2
                                ),
                                pair_rhs(q, kw, t),
                                start=(k == 0),
                                stop=(k == nmm - 1),
                                perf_mode=mybir.MatmulPerfMode.DoubleRow,
                            )
                            k += 1
                else:
                    k = 0
                    for i in range(2):  # A (slot q), B (slot q+1)
                        for kw in range(KW):
                            nc.tensor.matmul(
                                ps[:],
                                WTILES[i * 3 + kw][:],
                                slot_rhs(q + i, kw),
                                start=(k == 0),
                                stop=(k == nmm - 1),
                            )
                            k += 1
                evict(ogv, j, ps)
        if STORES:
            half_g = OG // 2
            nc.sync.dma_start(
                outv[:, OG * g : OG * g + half_g, :], ogv[:, 0:half_g, :]
            )
            nc.sync.dma_start(
                outv[:, OG * g + half_g : OG * (g + 1), :], ogv[:, half_g:OG, :]
            )

    for g in range(NOG):
        ensure_loaded(OG * (g + 1) + 1 + PREFETCH)
        compute_group(g)

    pools = [pmain, opool, const]
    if not cfg["resident"]:
        pools.insert(0, xpool)
    for p in pools:
        p.release()


def _get_nc(reps=1):
    if reps not in _CACHE:
        _CACHE[reps] = _build_nc(reps)
    return _CACHE[reps]


def _rn8(a):
    return a.astype(NPF8)


def _host_weights(weight, w, affine_w, affine_b):
    """Per-sample modulated+demodulated stationary tiles, quantized."""
    weight = np.asarray(weight, dtype=np.float64)
    w = np.asarray(w, dtype=np.float64)
    affine_w = np.asarray(affine_w, dtype=np.float64)
    affine_b = np.asarray(affine_b, dtype=np.float64)
    cfg = _mode_cfg()
    nmm, planes = cfg["nmm"], cfg["planes"]
    npdt = {
        F32R: np.float32,
        BF16: ml_dtypes.bfloat16,
        mybir.dt.float16: np.float16,
        FP8: NPF8,
    }[cfg["sdt"]]
    per_sample = []
    for b in range(B):
        s = w[b] @ affine_w.T + affine_b + 1.0  # [CIN]
        wm = s[None, :, None, None] * weight  # [co, ci, kh, kw]
        d = 1.0 / np.sqrt((wm * wm).sum(axis=(1, 2, 3)) + EPS)  # [co]
        wd = (d[:, None, None, None] * wm).transpose(1, 0, 2, 3)  # [ci, co, kh, kw]
        tiles = np.zeros((nmm, 128, planes * 128), dtype=npdt)
        if MODE != "fp8dr":
            wq = wd.astype(np.float32)
            for i in range(2):
                for kw in range(KW):
                    tl = tiles[i * 3 + kw]
                    for h in range(2):
                        for tau in range(2):
                            kh = h + 2 * i - tau
                            if 0 <= kh < KH:
                                tl[
                                    h * 64 : h * 64 + 64, tau * 64 : tau * 64 + 64
                                ] = wq[:, :, kh, kw]
        else:
            wq = (wd * WSCALE).astype(np.float32)
            wh = _rn8(wq)
            wl = _rn8(wq - wh.astype(np.float32))
            tv = tiles.reshape(nmm, 128, 2, 128)
            for t in range(3):
                w8 = wh if t < 2 else wl
                for kw in range(KW):
                    tl = tv[t * 3 + kw]
                    for h in range(2):
                        for i in range(2):
                            for tau in range(2):
                                kh = h + 2 * i - tau
                                if 0 <= kh < KH:
                                    tl[
                                        h * 64 : h * 64 + 64,
                                        i,
                                        tau * 64 : tau * 64 + 64,
                                    ] = w8[:, :, kh, kw]
        per_sample.append(tiles)
    return per_sample


def _host_x(x):
    """Slot-layout, padded, quantized per-core x tensors."""
    x = np.asarray(x, dtype=np.float32)
    cfg = _mode_cfg()
    npdt = {
        F32R: np.float32,
        BF16: ml_dtypes.bfloat16,
        mybir.dt.float16: np.float16,
        FP8: NPF8,
    }[cfg["xdt"]]
    shards = []
    for core in range(N_CORES):
        b, half = divmod(core, 2)
        h0 = half * HALF
        xsh = np.zeros((CIN, 2 * NSLOT, W), dtype=np.float32)
        lo, hi = h0 - 1, h0 + HALF + 1  # global rows [lo, hi)
        clo, chi = max(lo, 0), min(hi, H)
        xsh[:, clo - lo : chi - lo, :] = x[b, :, clo:chi, :]
        # xsh index idx = local row R + 1, R in -1..256
        # slot m: h=0 -> R=2m-1 (idx 2m), h=1 -> R=2m (idx 2m+1)
        xs = np.zeros((2, CIN, NSLOT, ROWB), dtype=np.float32)
        xs[0, :, :, 1:513] = xsh[:, 0 : 2 * NSLOT : 2, :]
        xs[1, :, :, 1:513] = xsh[:, 1 : 2 * NSLOT : 2, :]
        xs = xs.reshape(128, NSLOT * ROWB)
        if MODE == "fp8dr":
            xh = _rn8(xs)
            xl = _rn8(xs - xh.astype(np.float32))
            shards.append((xh, xl))
        else:
            shards.append((xs.astype(npdt), None))
    return shards


def _shard_inputs(x, w, weight, affine_w, affine_b):
    wts = _host_weights(weight, w, affine_w, affine_b)
    xsh = _host_x(x)
    in_maps = []
    for core in range(N_CORES):
        b = core // 2
        m = {"x1": xsh[core][0], "wtl": wts[b]}
        if MODE == "fp8dr":
            m["x2"] = xsh[core][1]
        in_maps.append(m)
    return in_maps


def kernel(x, w, weight, affine_w, affine_b):
    nc = _get_nc()
    in_maps = _shard_inputs(x, w, weight, affine_w, affine_b)
    res = run_bass_kernel_spmd(nc, in_maps, list(range(N_CORES)))
    full = np.empty((B, COUT, H, W), dtype=np.float32)
    for core in range(N_CORES):
        b, half = divmod(core, 2)
        o2 = res.results[core]["out"].astype(np.float32)  # [2, COUT, NPAIR, W]
        full[b, :, half * HALF : (half + 1) * HALF, :] = (
            o2.transpose(1, 2, 0, 3).reshape(COUT, HALF, W)
        )
    return full
